# revision 25
# baseline (speedup 1.0000x reference)
"""Multi-head self-attention on 8 Trainium2 NeuronCores.

Sharding: core c handles batch b = c//4 and head-group g = c%4 (4 of 16 heads,
feature slice [256g, 256(g+1))). QKV projections are computed per-core for its
head slice from a host-transposed activation x^T (so Q^T/K^T come out of the
TensorEngine directly in [head_dim, tokens] layout — no on-device transposes).
Attention uses the "augmented V" trick: scores are computed transposed
(S^T[k,q]), exponentiated on the ScalarEngine (scale folds in 1/sqrt(D)) two
heads per 1024-wide op, and the PV matmul with a ones-column appended to V
yields both the unnormalized context and the softmax denominator in one PSUM
accumulation. The combine_heads projection is row-sharded; partial outputs are
summed with an on-device ReduceScatter over each batch's 4 cores, chunked over
query ranges (smaller final chunks) so collectives overlap compute.

Compute dtype bf16 (fp32 accumulation in PSUM), output fp32.
"""

import sys

import numpy as np
import ml_dtypes

try:
    import concourse.bass as bass
except ImportError:  # fall back to known in-container locations
    for _p in ("/root/.axon_site/_ro/trn_rl_repo", "/opt/trn_rl_repo"):
        if _p not in sys.path:
            sys.path.append(_p)
    import concourse.bass as bass
import concourse.tile as tile
from concourse import bacc, mybir
from concourse.bass_utils import run_bass_kernel_spmd

B, S, E, H, D = 2, 2048, 1024, 16, 64
N_CORES = 8
GROUPS = 4            # head-groups (cores) per batch
FH = E // GROUPS      # 256 features (4 heads) per core
HL = FH // D          # 4 local heads
P = 128
EC = E // P           # 8 contraction chunks
SC = 1024             # wide psum width (two 512 banks)
QC = 512              # token-slice size for xT tiles / phase-1
NTS = S // QC         # 4
NT = S // P           # 16 key chunks of 128

# query-chunk plan: ReduceScatter chunk sizes (sum = S); smaller final
# chunks shrink the serialized collective tail
CHUNKS = [512, 512, 512, 512]
assert sum(CHUNKS) == S and all(c % 256 == 0 for c in CHUNKS)

F32 = mybir.dt.float32
BF16 = mybir.dt.bfloat16

_cached_nc = None


def _build():
    nc = bacc.Bacc("TRN2", target_bir_lowering=False, debug=False,
                   num_devices=N_CORES)

    xT = nc.dram_tensor("xT", [E, S], BF16, kind="ExternalInput")
    wqT = nc.dram_tensor("wqT", [E, FH], BF16, kind="ExternalInput")
    wkT = nc.dram_tensor("wkT", [E, FH], BF16, kind="ExternalInput")
    wvT = nc.dram_tensor("wvT", [E, FH], BF16, kind="ExternalInput")
    woT = nc.dram_tensor("woT", [FH, E], BF16, kind="ExternalInput")
    bq = nc.dram_tensor("bq", [FH], F32, kind="ExternalInput")
    bk = nc.dram_tensor("bk", [FH], F32, kind="ExternalInput")
    bv = nc.dram_tensor("bv", [FH], F32, kind="ExternalInput")
    bo4 = nc.dram_tensor("bo4", [E], F32, kind="ExternalInput")
    y = nc.dram_tensor("y", [S // GROUPS, E], F32, kind="ExternalOutput")

    with tile.TileContext(nc) as tc:
        with tc.tile_pool(name="const", bufs=1) as const, \
             tc.tile_pool(name="qpool", bufs=2) as qpool, \
             tc.tile_pool(name="epool", bufs=6) as epool, \
             tc.tile_pool(name="cpool", bufs=2) as cpool, \
             tc.tile_pool(name="rzpool", bufs=3) as rzpool, \
             tc.tile_pool(name="opool", bufs=3) as opool, \
             tc.tile_pool(name="psA", bufs=2, space="PSUM") as psA, \
             tc.tile_pool(name="psS", bufs=2, space="PSUM") as psS, \
             tc.tile_pool(name="psT", bufs=2, space="PSUM") as psT, \
             tc.tile_pool(name="dram", bufs=1, space="DRAM") as dram:

            # ---- DMAs: K weights first, then x slices, then the rest ----
            wq_sb = const.tile([P, EC, FH], BF16, tag="wq")
            wk_sb = const.tile([P, EC, FH], BF16, tag="wk")
            wv_sb = const.tile([P, EC, FH], BF16, tag="wv")
            wo_sb = const.tile([P, FH // P, E], BF16, tag="wo")
            xT_r = xT.ap().rearrange("(o p) t -> p o t", p=P)
            xTt = [const.tile([P, EC, QC], BF16, tag=f"xT{ts}",
                              name=f"xT{ts}") for ts in range(NTS)]

            def dma_x(ts, ep):
                # separate HWDGE queue (ACT) so x streams in parallel with
                # the weight DMAs on the SP queue; ACT is idle at load time
                nc.scalar.dma_start(
                    xTt[ts][:, 2 * ep:2 * ep + 2, :],
                    xT_r[:, 2 * ep:2 * ep + 2, ts * QC:(ts + 1) * QC])

            nc.sync.dma_start(wk_sb[:], wkT.ap().rearrange("(o p) f -> p o f", p=P))
            for ep in range(EC // 2):
                dma_x(0, ep)
            nc.sync.dma_start(wv_sb[:], wvT.ap().rearrange("(o p) f -> p o f", p=P))
            for ep in range(EC // 2):
                dma_x(1, ep)
            nc.sync.dma_start(wq_sb[:], wqT.ap().rearrange("(o p) f -> p o f", p=P))
            for ts in range(2, NTS):
                for ep in range(EC // 2):
                    dma_x(ts, ep)
            nc.sync.dma_start(wo_sb[:], woT.ap().rearrange("(o p) e -> p o e", p=P))

            bq_sb = const.tile([P, FH // P], F32, tag="bq")
            bk_sb = const.tile([P, FH // P], F32, tag="bk")
            nc.sync.dma_start(bk_sb[:], bk.ap().rearrange("(o p) -> p o", p=P))
            nc.sync.dma_start(bq_sb[:], bq.ap().rearrange("(o p) -> p o", p=P))

            def _pbcast(ap):
                return bass.AP(tensor=ap.tensor, offset=ap.offset,
                               ap=[[0, P], *ap.ap])

            bvb = const.tile([P, FH], F32, tag="bvb")
            nc.sync.dma_start(bvb[:], _pbcast(bv.ap()))
            bo4b = const.tile([P, E], F32, tag="bo4b")
            nc.sync.dma_start(bo4b[:], _pbcast(bo4.ap()))

            kT = const.tile([P, FH // P, S], BF16, tag="kT")
            vaug = const.tile([P, NT, HL, D + 1], BF16, tag="vaug")
            nc.vector.memset(vaug[:], 1.0)

            # ---- phase-1 pieces ----
            def emit_kT(fc, sc):
                """K^T for feature chunk fc, tokens [sc*SC, (sc+1)*SC)."""
                ps = psS.tile([P, SC], F32, tag="psS", name=f"kps{fc}{sc}")
                for half in range(SC // QC):
                    ts = sc * (SC // QC) + half
                    for e in range(EC):
                        nc.tensor.matmul(
                            ps[:, half * QC:(half + 1) * QC],
                            lhsT=wk_sb[:, e, fc * P:(fc + 1) * P],
                            rhs=xTt[ts][:, e, :],
                            start=(e == 0), stop=(e == EC - 1))
                nc.vector.tensor_scalar_add(kT[:, fc, sc * SC:(sc + 1) * SC],
                                            ps[:], bk_sb[:, fc:fc + 1])

            def emit_V(t):
                """V rows for t-chunk t (128 tokens, all heads) + bias.

                Uses the attention-accumulator psum slots (same tag) so the
                psS slots stay free for the S^T/exp pipeline to start early."""
                ps = psT.tile([P, FH], F32, tag="pattn", name=f"vps{t}")
                ts, off = t // (QC // P), (t % (QC // P)) * P
                for e in range(EC):
                    nc.tensor.matmul(
                        ps[:],
                        lhsT=xTt[ts][:, e, off:off + P],
                        rhs=wv_sb[:, e, :],
                        start=(e == 0), stop=(e == EC - 1))
                nc.vector.tensor_add(
                    vaug[:, t, :, 0:D],
                    ps[:].rearrange("p (h d) -> p h d", h=HL),
                    bvb[:].rearrange("p (h d) -> p h d", h=HL))

            def emit_phase1(sc):
                for fc in range(FH // P):
                    emit_kT(fc, sc)
                for t in range(sc * (SC // P), (sc + 1) * (SC // P)):
                    emit_V(t)

            # ---- phase-2 pieces ----
            def emit_qT(ci, q0, w):
                """Q^T for tokens [q0, q0+w); w in {256, 512}."""
                qT = qpool.tile([P, FH // P, QC], BF16, tag="qT",
                                name=f"qT{ci}")
                for fc in range(FH // P):
                    ps = psA.tile([P, 512], F32, tag="psA", name=f"qps{fc}")
                    for off in range(0, w, QC):
                        ts, o2 = divmod(q0 + off, QC)
                        ww = min(QC - o2, w - off)
                        for e in range(EC):
                            nc.tensor.matmul(
                                ps[:, off:off + ww],
                                lhsT=wq_sb[:, e, fc * P:(fc + 1) * P],
                                rhs=xTt[ts][:, e, o2:o2 + ww],
                                start=(e == 0), stop=(e == EC - 1))
                    nc.vector.tensor_scalar_add(qT[:, fc, :w],
                                                ps[:, :w],
                                                bq_sb[:, fc:fc + 1])
                return qT

            def emit_attention(qT, cT, w):
                for hc in range(HL // 2):  # head pair (2*hc, 2*hc+1)
                    pattn = [psT.tile([D + 1, QC], F32, tag="pattn",
                                      name=f"pattn{hp}") for hp in range(2)]
                    for k in range(NT):
                        pss = psS.tile([P, SC], F32, tag="psS", name="pss")
                        for hp in range(2):
                            hs = slice(hp * D, (hp + 1) * D)
                            nc.tensor.matmul(
                                pss[:, hp * QC:hp * QC + w],
                                lhsT=kT[hs, hc, k * P:(k + 1) * P],
                                rhs=qT[hs, hc, :w],
                                start=True, stop=True)
                        et = epool.tile([P, SC], BF16, tag="et")
                        if w == QC:
                            nc.scalar.activation(
                                et[:], pss[:],
                                mybir.ActivationFunctionType.Exp, scale=0.125)
                        else:
                            for hp in range(2):
                                nc.scalar.activation(
                                    et[:, hp * QC:hp * QC + w],
                                    pss[:, hp * QC:hp * QC + w],
                                    mybir.ActivationFunctionType.Exp,
                                    scale=0.125)
                        for hp in range(2):
                            nc.tensor.matmul(
                                pattn[hp][:, :w],
                                lhsT=vaug[:, k, 2 * hc + hp, :],
                                rhs=et[:, hp * QC:hp * QC + w],
                                start=(k == 0), stop=(k == NT - 1))
                    for hp in range(2):
                        hs = slice(hp * D, (hp + 1) * D)
                        # drain PSUM to SBUF first so the accumulator slot
                        # frees for the next head pair right away
                        nsb = rzpool.tile([D + 1, QC], F32, tag="nsb")
                        nc.vector.tensor_copy(nsb[:, :w], pattn[hp][:, :w])
                        rz = rzpool.tile([1, QC], F32, tag="rz")
                        nc.vector.reciprocal(rz[:, :w], nsb[D:D + 1, :w])
                        rzb = rzpool.tile([D, QC], F32, tag="rzb")
                        nc.gpsimd.partition_broadcast(rzb[:, :w], rz[0:1, :w])
                        nc.vector.tensor_mul(cT[hs, hc, :w],
                                             nsb[0:D, :w], rzb[:, :w])

            def emit_outproj_rs(ci, cT, q0, w):
                partial = dram.tile([w, E], F32, tag=f"partial{ci}")
                for t4 in range(w // P):
                    pso = [psA.tile([P, 512], F32, tag="psA",
                                    name=f"pso{eh}") for eh in range(2)]
                    for eh in range(2):
                        for fc in range(FH // P):
                            nc.tensor.matmul(
                                pso[eh][:],
                                lhsT=cT[:, fc, t4 * P:(t4 + 1) * P],
                                rhs=wo_sb[:, fc, eh * 512:(eh + 1) * 512],
                                start=(fc == 0), stop=(fc == FH // P - 1))
                    outsb = opool.tile([P, E], F32, tag="outsb")
                    for eh in range(2):
                        nc.vector.tensor_add(
                            outsb[:, eh * 512:(eh + 1) * 512],
                            pso[eh][:],
                            bo4b[:, eh * 512:(eh + 1) * 512])
                    nc.sync.dma_start(partial[t4 * P:(t4 + 1) * P, :],
                                      outsb[:])

                rsout = dram.tile([w // GROUPS, E], F32, tag=f"rsout{ci}")
                nc.gpsimd.collective_compute(
                    "ReduceScatter",
                    mybir.AluOpType.add,
                    replica_groups=[[0, 1, 2, 3], [4, 5, 6, 7]],
                    ins=[partial.opt()],
                    outs=[rsout.opt()],
                )
                nc.sync.dma_start(y.ap()[q0 // GROUPS:(q0 + w) // GROUPS, :],
                                  rsout[:])

            # ---- emission order: out-proj of chunk ci-1 is emitted after
            # attention of chunk ci, so its matmuls fill the PE-idle slots of
            # the ACT-bound attention inner loop ----
            emit_phase1(0)
            pending = None  # (ci, cT, q0, w) awaiting out-proj
            q0 = 0
            for ci, w in enumerate(CHUNKS):
                qT = emit_qT(ci, q0, w)
                if ci == 0:
                    emit_phase1(1)
                cT = cpool.tile([P, FH // P, QC], BF16, tag="cT",
                                name=f"cT{ci}")
                emit_attention(qT, cT, w)
                if pending is not None:
                    emit_outproj_rs(*pending)
                pending = (ci, cT, q0, w)
                q0 += w
            emit_outproj_rs(*pending)

    nc.compile()
    return nc


def _get_nc():
    global _cached_nc
    if _cached_nc is None:
        _cached_nc = _build()
    return _cached_nc


def _in_maps(inputs, Wq, bq, Wk, bk, Wv, bv, Wo, bo):
    x = np.asarray(inputs, dtype=np.float32)
    xTb = [np.ascontiguousarray(x[b].T).astype(ml_dtypes.bfloat16)
           for b in range(B)]
    maps = []
    for c in range(N_CORES):
        b, g = divmod(c, GROUPS)
        fs, fe = g * FH, (g + 1) * FH
        maps.append({
            "xT": xTb[b],
            "wqT": np.ascontiguousarray(np.asarray(Wq)[fs:fe, :].T).astype(ml_dtypes.bfloat16),
            "wkT": np.ascontiguousarray(np.asarray(Wk)[fs:fe, :].T).astype(ml_dtypes.bfloat16),
            "wvT": np.ascontiguousarray(np.asarray(Wv)[fs:fe, :].T).astype(ml_dtypes.bfloat16),
            "woT": np.ascontiguousarray(np.asarray(Wo)[:, fs:fe].T).astype(ml_dtypes.bfloat16),
            "bq": np.ascontiguousarray(np.asarray(bq)[fs:fe], dtype=np.float32),
            "bk": np.ascontiguousarray(np.asarray(bk)[fs:fe], dtype=np.float32),
            "bv": np.ascontiguousarray(np.asarray(bv)[fs:fe], dtype=np.float32),
            "bo4": (np.asarray(bo, dtype=np.float32) / GROUPS).copy(),
        })
    return maps


def _assemble(results):
    out = np.empty((B, S, E), dtype=np.float32)
    for c in range(N_CORES):
        b, p = divmod(c, GROUPS)
        yc = results[c]["y"]  # [S//GROUPS, E]
        q0 = 0
        for w in CHUNKS:
            rows = w // GROUPS
            src = (q0 // GROUPS)
            out[b, q0 + p * rows:q0 + (p + 1) * rows, :] = yc[src:src + rows]
            q0 += w
    return out


def kernel(inputs, Wq, bq, Wk, bk, Wv, bv, Wo, bo, _run_kwargs=None):
    nc = _get_nc()
    maps = _in_maps(inputs, Wq, bq, Wk, bk, Wv, bv, Wo, bo)
    res = run_bass_kernel_spmd(nc, maps, core_ids=list(range(N_CORES)),
                               **(_run_kwargs or {}))
    if _run_kwargs:
        kernel.last_results = res
    return _assemble(res.results)


# revision 26
# speedup vs baseline: 1.0145x; 1.0145x over previous
"""Multi-head self-attention on 8 Trainium2 NeuronCores.

Sharding: core c handles batch b = c//4 and head-group g = c%4 (4 of 16 heads,
feature slice [256g, 256(g+1))). QKV projections are computed per-core for its
head slice from a host-transposed activation x^T (so Q^T/K^T come out of the
TensorEngine directly in [head_dim, tokens] layout — no on-device transposes).
Attention uses the "augmented V" trick: scores are computed transposed
(S^T[k,q]), exponentiated on the ScalarEngine (scale folds in 1/sqrt(D)) two
heads per 1024-wide op, and the PV matmul with a ones-column appended to V
yields both the unnormalized context and the softmax denominator in one PSUM
accumulation. The combine_heads projection is row-sharded; partial outputs are
summed with an on-device ReduceScatter over each batch's 4 cores, chunked over
query ranges (smaller final chunks) so collectives overlap compute.

Compute dtype bf16 (fp32 accumulation in PSUM), output fp32.
"""

import sys

import numpy as np
import ml_dtypes

try:
    import concourse.bass as bass
except ImportError:  # fall back to known in-container locations
    for _p in ("/root/.axon_site/_ro/trn_rl_repo", "/opt/trn_rl_repo"):
        if _p not in sys.path:
            sys.path.append(_p)
    import concourse.bass as bass
import concourse.tile as tile
from concourse import bacc, mybir
from concourse.bass_utils import run_bass_kernel_spmd

B, S, E, H, D = 2, 2048, 1024, 16, 64
N_CORES = 8
GROUPS = 4            # head-groups (cores) per batch
FH = E // GROUPS      # 256 features (4 heads) per core
HL = FH // D          # 4 local heads
P = 128
EC = E // P           # 8 contraction chunks
SC = 1024             # wide psum width (two 512 banks)
QC = 512              # token-slice size for xT tiles / phase-1
NTS = S // QC         # 4
NT = S // P           # 16 key chunks of 128

# query-chunk plan: ReduceScatter chunk sizes (sum = S); smaller final
# chunks shrink the serialized collective tail
CHUNKS = [512, 512, 512, 512]
assert sum(CHUNKS) == S and all(c % 256 == 0 for c in CHUNKS)

F32 = mybir.dt.float32
BF16 = mybir.dt.bfloat16

_cached_nc = None


def _build():
    nc = bacc.Bacc("TRN2", target_bir_lowering=False, debug=False,
                   num_devices=N_CORES)

    xT = nc.dram_tensor("xT", [E, S], BF16, kind="ExternalInput")
    wqT = nc.dram_tensor("wqT", [E, FH], BF16, kind="ExternalInput")
    wkT = nc.dram_tensor("wkT", [E, FH], BF16, kind="ExternalInput")
    wvT = nc.dram_tensor("wvT", [E, FH], BF16, kind="ExternalInput")
    woT = nc.dram_tensor("woT", [FH, E], BF16, kind="ExternalInput")
    bq = nc.dram_tensor("bq", [FH], F32, kind="ExternalInput")
    bk = nc.dram_tensor("bk", [FH], F32, kind="ExternalInput")
    bv = nc.dram_tensor("bv", [FH], F32, kind="ExternalInput")
    bo4 = nc.dram_tensor("bo4", [E], F32, kind="ExternalInput")
    y = nc.dram_tensor("y", [S // GROUPS, E], F32, kind="ExternalOutput")

    with tile.TileContext(nc) as tc:
        with tc.tile_pool(name="const", bufs=1) as const, \
             tc.tile_pool(name="qpool", bufs=2) as qpool, \
             tc.tile_pool(name="epool", bufs=8) as epool, \
             tc.tile_pool(name="cpool", bufs=2) as cpool, \
             tc.tile_pool(name="rzpool", bufs=4) as rzpool, \
             tc.tile_pool(name="opool", bufs=4) as opool, \
             tc.tile_pool(name="psA", bufs=2, space="PSUM") as psA, \
             tc.tile_pool(name="psS", bufs=2, space="PSUM") as psS, \
             tc.tile_pool(name="psT", bufs=2, space="PSUM") as psT, \
             tc.tile_pool(name="dram", bufs=1, space="DRAM") as dram:

            # ---- DMAs: K weights first, then x slices, then the rest ----
            wq_sb = const.tile([P, EC, FH], BF16, tag="wq")
            wk_sb = const.tile([P, EC, FH], BF16, tag="wk")
            wv_sb = const.tile([P, EC, FH], BF16, tag="wv")
            wo_sb = const.tile([P, FH // P, E], BF16, tag="wo")
            xT_r = xT.ap().rearrange("(o p) t -> p o t", p=P)
            xTt = [const.tile([P, EC, QC], BF16, tag=f"xT{ts}",
                              name=f"xT{ts}") for ts in range(NTS)]

            def dma_x(ts, ep):
                # separate HWDGE queue (ACT) so x streams in parallel with
                # the weight DMAs on the SP queue; ACT is idle at load time
                nc.scalar.dma_start(
                    xTt[ts][:, 2 * ep:2 * ep + 2, :],
                    xT_r[:, 2 * ep:2 * ep + 2, ts * QC:(ts + 1) * QC])

            nc.sync.dma_start(wk_sb[:], wkT.ap().rearrange("(o p) f -> p o f", p=P))
            for ep in range(EC // 2):
                dma_x(0, ep)
            nc.sync.dma_start(wv_sb[:], wvT.ap().rearrange("(o p) f -> p o f", p=P))
            for ep in range(EC // 2):
                dma_x(1, ep)
            nc.sync.dma_start(wq_sb[:], wqT.ap().rearrange("(o p) f -> p o f", p=P))
            for ts in range(2, NTS):
                for ep in range(EC // 2):
                    dma_x(ts, ep)
            nc.sync.dma_start(wo_sb[:], woT.ap().rearrange("(o p) e -> p o e", p=P))

            bq_sb = const.tile([P, FH // P], F32, tag="bq")
            bk_sb = const.tile([P, FH // P], F32, tag="bk")
            nc.sync.dma_start(bk_sb[:], bk.ap().rearrange("(o p) -> p o", p=P))
            nc.sync.dma_start(bq_sb[:], bq.ap().rearrange("(o p) -> p o", p=P))

            def _pbcast(ap):
                return bass.AP(tensor=ap.tensor, offset=ap.offset,
                               ap=[[0, P], *ap.ap])

            bvb = const.tile([P, FH], F32, tag="bvb")
            nc.sync.dma_start(bvb[:], _pbcast(bv.ap()))
            bo4b = const.tile([P, E], F32, tag="bo4b")
            nc.sync.dma_start(bo4b[:], _pbcast(bo4.ap()))

            kT = const.tile([P, FH // P, S], BF16, tag="kT")
            vaug = const.tile([P, NT, HL, D + 1], BF16, tag="vaug")
            nc.vector.memset(vaug[:], 1.0)

            # ---- phase-1 pieces ----
            def emit_kT(fc, sc):
                """K^T for feature chunk fc, tokens [sc*SC, (sc+1)*SC)."""
                ps = psS.tile([P, SC], F32, tag="psS", name=f"kps{fc}{sc}")
                for half in range(SC // QC):
                    ts = sc * (SC // QC) + half
                    for e in range(EC):
                        nc.tensor.matmul(
                            ps[:, half * QC:(half + 1) * QC],
                            lhsT=wk_sb[:, e, fc * P:(fc + 1) * P],
                            rhs=xTt[ts][:, e, :],
                            start=(e == 0), stop=(e == EC - 1))
                nc.vector.tensor_scalar_add(kT[:, fc, sc * SC:(sc + 1) * SC],
                                            ps[:], bk_sb[:, fc:fc + 1])

            def emit_V(t):
                """V rows for t-chunk t (128 tokens, all heads) + bias.

                Uses the attention-accumulator psum slots (same tag) so the
                psS slots stay free for the S^T/exp pipeline to start early."""
                ps = psT.tile([P, FH], F32, tag="pattn", name=f"vps{t}")
                ts, off = t // (QC // P), (t % (QC // P)) * P
                for e in range(EC):
                    nc.tensor.matmul(
                        ps[:],
                        lhsT=xTt[ts][:, e, off:off + P],
                        rhs=wv_sb[:, e, :],
                        start=(e == 0), stop=(e == EC - 1))
                nc.vector.tensor_add(
                    vaug[:, t, :, 0:D],
                    ps[:].rearrange("p (h d) -> p h d", h=HL),
                    bvb[:].rearrange("p (h d) -> p h d", h=HL))

            def emit_phase1(sc):
                for fc in range(FH // P):
                    emit_kT(fc, sc)
                for t in range(sc * (SC // P), (sc + 1) * (SC // P)):
                    emit_V(t)

            # ---- phase-2 pieces ----
            def emit_qT(ci, q0, w):
                """Q^T for tokens [q0, q0+w); w in {256, 512}."""
                qT = qpool.tile([P, FH // P, QC], BF16, tag="qT",
                                name=f"qT{ci}")
                for fc in range(FH // P):
                    ps = psA.tile([P, 512], F32, tag="psA", name=f"qps{fc}")
                    for off in range(0, w, QC):
                        ts, o2 = divmod(q0 + off, QC)
                        ww = min(QC - o2, w - off)
                        for e in range(EC):
                            nc.tensor.matmul(
                                ps[:, off:off + ww],
                                lhsT=wq_sb[:, e, fc * P:(fc + 1) * P],
                                rhs=xTt[ts][:, e, o2:o2 + ww],
                                start=(e == 0), stop=(e == EC - 1))
                    nc.vector.tensor_scalar_add(qT[:, fc, :w],
                                                ps[:, :w],
                                                bq_sb[:, fc:fc + 1])
                return qT

            def emit_attention(qT, cT, w):
                for hc in range(HL // 2):  # head pair (2*hc, 2*hc+1)
                    pattn = [psT.tile([D + 1, QC], F32, tag="pattn",
                                      name=f"pattn{hp}") for hp in range(2)]
                    for k in range(NT):
                        pss = psS.tile([P, SC], F32, tag="psS", name="pss")
                        for hp in range(2):
                            hs = slice(hp * D, (hp + 1) * D)
                            nc.tensor.matmul(
                                pss[:, hp * QC:hp * QC + w],
                                lhsT=kT[hs, hc, k * P:(k + 1) * P],
                                rhs=qT[hs, hc, :w],
                                start=True, stop=True)
                        et = epool.tile([P, SC], BF16, tag="et")
                        if w == QC:
                            nc.scalar.activation(
                                et[:], pss[:],
                                mybir.ActivationFunctionType.Exp, scale=0.125)
                        else:
                            for hp in range(2):
                                nc.scalar.activation(
                                    et[:, hp * QC:hp * QC + w],
                                    pss[:, hp * QC:hp * QC + w],
                                    mybir.ActivationFunctionType.Exp,
                                    scale=0.125)
                        for hp in range(2):
                            nc.tensor.matmul(
                                pattn[hp][:, :w],
                                lhsT=vaug[:, k, 2 * hc + hp, :],
                                rhs=et[:, hp * QC:hp * QC + w],
                                start=(k == 0), stop=(k == NT - 1))
                    for hp in range(2):
                        hs = slice(hp * D, (hp + 1) * D)
                        # drain PSUM to SBUF first so the accumulator slot
                        # frees for the next head pair right away
                        nsb = rzpool.tile([D + 1, QC], F32, tag="nsb")
                        nc.vector.tensor_copy(nsb[:, :w], pattn[hp][:, :w])
                        rz = rzpool.tile([1, QC], F32, tag="rz")
                        nc.vector.reciprocal(rz[:, :w], nsb[D:D + 1, :w])
                        rzb = rzpool.tile([D, QC], F32, tag="rzb")
                        nc.gpsimd.partition_broadcast(rzb[:, :w], rz[0:1, :w])
                        nc.vector.tensor_mul(cT[hs, hc, :w],
                                             nsb[0:D, :w], rzb[:, :w])

            def emit_outproj_rs(ci, cT, q0, w):
                partial = dram.tile([w, E], F32, tag=f"partial{ci}")
                for t4 in range(w // P):
                    pso = [psA.tile([P, 512], F32, tag="psA",
                                    name=f"pso{eh}") for eh in range(2)]
                    for eh in range(2):
                        for fc in range(FH // P):
                            nc.tensor.matmul(
                                pso[eh][:],
                                lhsT=cT[:, fc, t4 * P:(t4 + 1) * P],
                                rhs=wo_sb[:, fc, eh * 512:(eh + 1) * 512],
                                start=(fc == 0), stop=(fc == FH // P - 1))
                    outsb = opool.tile([P, E], F32, tag="outsb")
                    for eh in range(2):
                        nc.vector.tensor_add(
                            outsb[:, eh * 512:(eh + 1) * 512],
                            pso[eh][:],
                            bo4b[:, eh * 512:(eh + 1) * 512])
                    nc.sync.dma_start(partial[t4 * P:(t4 + 1) * P, :],
                                      outsb[:])

                rsout = dram.tile([w // GROUPS, E], F32, tag=f"rsout{ci}")
                nc.gpsimd.collective_compute(
                    "ReduceScatter",
                    mybir.AluOpType.add,
                    replica_groups=[[0, 1, 2, 3], [4, 5, 6, 7]],
                    ins=[partial.opt()],
                    outs=[rsout.opt()],
                )
                nc.sync.dma_start(y.ap()[q0 // GROUPS:(q0 + w) // GROUPS, :],
                                  rsout[:])

            # ---- emission order: out-proj of chunk ci-1 is emitted after
            # attention of chunk ci, so its matmuls fill the PE-idle slots of
            # the ACT-bound attention inner loop ----
            emit_phase1(0)
            pending = None  # (ci, cT, q0, w) awaiting out-proj
            q0 = 0
            for ci, w in enumerate(CHUNKS):
                qT = emit_qT(ci, q0, w)
                if ci == 0:
                    emit_phase1(1)
                cT = cpool.tile([P, FH // P, QC], BF16, tag="cT",
                                name=f"cT{ci}")
                emit_attention(qT, cT, w)
                if pending is not None:
                    emit_outproj_rs(*pending)
                pending = (ci, cT, q0, w)
                q0 += w
            emit_outproj_rs(*pending)

    nc.compile()
    return nc


def _get_nc():
    global _cached_nc
    if _cached_nc is None:
        _cached_nc = _build()
    return _cached_nc


def _in_maps(inputs, Wq, bq, Wk, bk, Wv, bv, Wo, bo):
    x = np.asarray(inputs, dtype=np.float32)
    xTb = [np.ascontiguousarray(x[b].T).astype(ml_dtypes.bfloat16)
           for b in range(B)]
    maps = []
    for c in range(N_CORES):
        b, g = divmod(c, GROUPS)
        fs, fe = g * FH, (g + 1) * FH
        maps.append({
            "xT": xTb[b],
            "wqT": np.ascontiguousarray(np.asarray(Wq)[fs:fe, :].T).astype(ml_dtypes.bfloat16),
            "wkT": np.ascontiguousarray(np.asarray(Wk)[fs:fe, :].T).astype(ml_dtypes.bfloat16),
            "wvT": np.ascontiguousarray(np.asarray(Wv)[fs:fe, :].T).astype(ml_dtypes.bfloat16),
            "woT": np.ascontiguousarray(np.asarray(Wo)[:, fs:fe].T).astype(ml_dtypes.bfloat16),
            "bq": np.ascontiguousarray(np.asarray(bq)[fs:fe], dtype=np.float32),
            "bk": np.ascontiguousarray(np.asarray(bk)[fs:fe], dtype=np.float32),
            "bv": np.ascontiguousarray(np.asarray(bv)[fs:fe], dtype=np.float32),
            "bo4": (np.asarray(bo, dtype=np.float32) / GROUPS).copy(),
        })
    return maps


def _assemble(results):
    out = np.empty((B, S, E), dtype=np.float32)
    for c in range(N_CORES):
        b, p = divmod(c, GROUPS)
        yc = results[c]["y"]  # [S//GROUPS, E]
        q0 = 0
        for w in CHUNKS:
            rows = w // GROUPS
            src = (q0 // GROUPS)
            out[b, q0 + p * rows:q0 + (p + 1) * rows, :] = yc[src:src + rows]
            q0 += w
    return out


def kernel(inputs, Wq, bq, Wk, bk, Wv, bv, Wo, bo, _run_kwargs=None):
    nc = _get_nc()
    maps = _in_maps(inputs, Wq, bq, Wk, bk, Wv, bv, Wo, bo)
    res = run_bass_kernel_spmd(nc, maps, core_ids=list(range(N_CORES)),
                               **(_run_kwargs or {}))
    if _run_kwargs:
        kernel.last_results = res
    return _assemble(res.results)


# revision 31
# speedup vs baseline: 1.0290x; 1.0142x over previous
"""Multi-head self-attention on 8 Trainium2 NeuronCores.

Sharding: core c handles batch b = c//4 and head-group g = c%4 (4 of 16 heads,
feature slice [256g, 256(g+1))). QKV projections are computed per-core for its
head slice from a host-transposed activation x^T (so Q^T/K^T come out of the
TensorEngine directly in [head_dim, tokens] layout — no on-device transposes).
Attention uses the "augmented V" trick: scores are computed transposed
(S^T[k,q]), exponentiated on the ScalarEngine (scale folds in 1/sqrt(D)) two
heads per 1024-wide op, and the PV matmul with a ones-column appended to V
yields both the unnormalized context and the softmax denominator in one PSUM
accumulation. The combine_heads projection is row-sharded; partial outputs are
summed with an on-device ReduceScatter over each batch's 4 cores, chunked over
query ranges (smaller final chunks) so collectives overlap compute.

Compute dtype bf16 (fp32 accumulation in PSUM), output fp32.
"""

import sys

import numpy as np
import ml_dtypes

try:
    import concourse.bass as bass
except ImportError:  # fall back to known in-container locations
    for _p in ("/root/.axon_site/_ro/trn_rl_repo", "/opt/trn_rl_repo"):
        if _p not in sys.path:
            sys.path.append(_p)
    import concourse.bass as bass
import concourse.tile as tile
from concourse import bacc, mybir
from concourse.bass_utils import run_bass_kernel_spmd

B, S, E, H, D = 2, 2048, 1024, 16, 64
N_CORES = 8
GROUPS = 4            # head-groups (cores) per batch
FH = E // GROUPS      # 256 features (4 heads) per core
HL = FH // D          # 4 local heads
P = 128
EC = E // P           # 8 contraction chunks
SC = 1024             # wide psum width (two 512 banks)
QC = 512              # token-slice size for xT tiles / phase-1
NTS = S // QC         # 4
NT = S // P           # 16 key chunks of 128

# query-chunk plan: ReduceScatter chunk sizes (sum = S); smaller final
# chunks shrink the serialized collective tail
CHUNKS = [512, 512, 512, 512]
assert sum(CHUNKS) == S and all(c % 256 == 0 for c in CHUNKS)

F32 = mybir.dt.float32
BF16 = mybir.dt.bfloat16

_cached_nc = None


def _build():
    nc = bacc.Bacc("TRN2", target_bir_lowering=False, debug=False,
                   num_devices=N_CORES)

    xT = nc.dram_tensor("xT", [E, S], BF16, kind="ExternalInput")
    wqT = nc.dram_tensor("wqT", [E, FH], BF16, kind="ExternalInput")
    wkT = nc.dram_tensor("wkT", [E, FH], BF16, kind="ExternalInput")
    wvT = nc.dram_tensor("wvT", [E, FH], BF16, kind="ExternalInput")
    woT = nc.dram_tensor("woT", [FH, E], BF16, kind="ExternalInput")
    bq = nc.dram_tensor("bq", [FH], F32, kind="ExternalInput")
    bk = nc.dram_tensor("bk", [FH], F32, kind="ExternalInput")
    bv = nc.dram_tensor("bv", [FH], F32, kind="ExternalInput")
    bo4 = nc.dram_tensor("bo4", [E], F32, kind="ExternalInput")
    y = nc.dram_tensor("y", [S // GROUPS, E], F32, kind="ExternalOutput")

    with tile.TileContext(nc) as tc:
        with tc.tile_pool(name="const", bufs=1) as const, \
             tc.tile_pool(name="qpool", bufs=2) as qpool, \
             tc.tile_pool(name="epool", bufs=8) as epool, \
             tc.tile_pool(name="cpool", bufs=2) as cpool, \
             tc.tile_pool(name="rzpool", bufs=4) as rzpool, \
             tc.tile_pool(name="opool", bufs=4) as opool, \
             tc.tile_pool(name="psA", bufs=2, space="PSUM") as psA, \
             tc.tile_pool(name="psS", bufs=2, space="PSUM") as psS, \
             tc.tile_pool(name="psT", bufs=2, space="PSUM") as psT, \
             tc.tile_pool(name="dram", bufs=1, space="DRAM") as dram:

            # ---- DMAs: K weights first, then x slices, then the rest ----
            wq_sb = const.tile([P, EC, FH], BF16, tag="wq")
            wk_sb = const.tile([P, EC, FH], BF16, tag="wk")
            wv_sb = const.tile([P, EC, FH], BF16, tag="wv")
            wo_sb = const.tile([P, FH // P, E], BF16, tag="wo")
            xT_r = xT.ap().rearrange("(o p) t -> p o t", p=P)
            xTt = [const.tile([P, EC, QC], BF16, tag=f"xT{ts}",
                              name=f"xT{ts}") for ts in range(NTS)]

            def dma_x(ts, ep):
                # separate HWDGE queue (ACT) so x streams in parallel with
                # the weight DMAs on the SP queue; ACT is idle at load time
                nc.scalar.dma_start(
                    xTt[ts][:, 2 * ep:2 * ep + 2, :],
                    xT_r[:, 2 * ep:2 * ep + 2, ts * QC:(ts + 1) * QC])

            nc.sync.dma_start(wk_sb[:], wkT.ap().rearrange("(o p) f -> p o f", p=P))
            for ep in range(EC // 2):
                dma_x(0, ep)
            nc.sync.dma_start(wv_sb[:], wvT.ap().rearrange("(o p) f -> p o f", p=P))
            for ep in range(EC // 2):
                dma_x(1, ep)
            nc.sync.dma_start(wq_sb[:], wqT.ap().rearrange("(o p) f -> p o f", p=P))
            for ts in range(2, NTS):
                for ep in range(EC // 2):
                    dma_x(ts, ep)
            nc.sync.dma_start(wo_sb[:], woT.ap().rearrange("(o p) e -> p o e", p=P))

            bq_sb = const.tile([P, FH // P], F32, tag="bq")
            bk_sb = const.tile([P, FH // P], F32, tag="bk")
            nc.sync.dma_start(bk_sb[:], bk.ap().rearrange("(o p) -> p o", p=P))
            nc.sync.dma_start(bq_sb[:], bq.ap().rearrange("(o p) -> p o", p=P))

            def _pbcast(ap):
                return bass.AP(tensor=ap.tensor, offset=ap.offset,
                               ap=[[0, P], *ap.ap])

            bvb = const.tile([P, FH], F32, tag="bvb")
            nc.sync.dma_start(bvb[:], _pbcast(bv.ap()))
            bo4b = const.tile([P, E], F32, tag="bo4b")
            nc.sync.dma_start(bo4b[:], _pbcast(bo4.ap()))

            kT = const.tile([P, FH // P, S], BF16, tag="kT")
            vaug = const.tile([P, NT, HL, D + 1], BF16, tag="vaug")
            nc.vector.memset(vaug[:], 1.0)

            # ---- phase-1 pieces ----
            def emit_kT(fc, sc):
                """K^T for feature chunk fc, tokens [sc*SC, (sc+1)*SC)."""
                ps = psS.tile([P, SC], F32, tag="psS", name=f"kps{fc}{sc}")
                for half in range(SC // QC):
                    ts = sc * (SC // QC) + half
                    for e in range(EC):
                        nc.tensor.matmul(
                            ps[:, half * QC:(half + 1) * QC],
                            lhsT=wk_sb[:, e, fc * P:(fc + 1) * P],
                            rhs=xTt[ts][:, e, :],
                            start=(e == 0), stop=(e == EC - 1))
                nc.vector.tensor_scalar_add(kT[:, fc, sc * SC:(sc + 1) * SC],
                                            ps[:], bk_sb[:, fc:fc + 1])

            def emit_V(t):
                """V rows for t-chunk t (128 tokens, all heads) + bias.

                Uses the attention-accumulator psum slots (same tag) so the
                psS slots stay free for the S^T/exp pipeline to start early."""
                ps = psA.tile([P, FH], F32, tag="psA", name=f"vps{t}")
                ts, off = t // (QC // P), (t % (QC // P)) * P
                for e in range(EC):
                    nc.tensor.matmul(
                        ps[:],
                        lhsT=xTt[ts][:, e, off:off + P],
                        rhs=wv_sb[:, e, :],
                        start=(e == 0), stop=(e == EC - 1))
                nc.vector.tensor_add(
                    vaug[:, t, :, 0:D],
                    ps[:].rearrange("p (h d) -> p h d", h=HL),
                    bvb[:].rearrange("p (h d) -> p h d", h=HL))

            def emit_phase1(sc):
                for fc in range(FH // P):
                    emit_kT(fc, sc)
                for t in range(sc * (SC // P), (sc + 1) * (SC // P)):
                    emit_V(t)

            # ---- phase-2 pieces ----
            def emit_qT(ci, q0, w):
                """Q^T for tokens [q0, q0+w); w in {256, 512}."""
                qT = qpool.tile([P, FH // P, QC], BF16, tag="qT",
                                name=f"qT{ci}")
                for fc in range(FH // P):
                    ps = psA.tile([P, 512], F32, tag="psA", name=f"qps{fc}")
                    for off in range(0, w, QC):
                        ts, o2 = divmod(q0 + off, QC)
                        ww = min(QC - o2, w - off)
                        for e in range(EC):
                            nc.tensor.matmul(
                                ps[:, off:off + ww],
                                lhsT=wq_sb[:, e, fc * P:(fc + 1) * P],
                                rhs=xTt[ts][:, e, o2:o2 + ww],
                                start=(e == 0), stop=(e == EC - 1))
                    nc.vector.tensor_scalar_add(qT[:, fc, :w],
                                                ps[:, :w],
                                                bq_sb[:, fc:fc + 1])
                return qT

            def emit_attention(qT, cT, w, mid=None):
                for hc in range(HL // 2):  # head pair (2*hc, 2*hc+1)
                    pattn = [psT.tile([D + 1, QC], F32, tag="pattn",
                                      name=f"pattn{hp}") for hp in range(2)]
                    for k in range(NT):
                        if mid is not None and hc == 0 and k == NT // 2:
                            mid()
                        pss = psS.tile([P, SC], F32, tag="psS", name="pss")
                        for hp in range(2):
                            hs = slice(hp * D, (hp + 1) * D)
                            nc.tensor.matmul(
                                pss[:, hp * QC:hp * QC + w],
                                lhsT=kT[hs, hc, k * P:(k + 1) * P],
                                rhs=qT[hs, hc, :w],
                                start=True, stop=True)
                        et = epool.tile([P, SC], BF16, tag="et")
                        if w == QC:
                            nc.scalar.activation(
                                et[:], pss[:],
                                mybir.ActivationFunctionType.Exp, scale=0.125)
                        else:
                            for hp in range(2):
                                nc.scalar.activation(
                                    et[:, hp * QC:hp * QC + w],
                                    pss[:, hp * QC:hp * QC + w],
                                    mybir.ActivationFunctionType.Exp,
                                    scale=0.125)
                        for hp in range(2):
                            nc.tensor.matmul(
                                pattn[hp][:, :w],
                                lhsT=vaug[:, k, 2 * hc + hp, :],
                                rhs=et[:, hp * QC:hp * QC + w],
                                start=(k == 0), stop=(k == NT - 1))
                    for hp in range(2):
                        hs = slice(hp * D, (hp + 1) * D)
                        # drain PSUM to SBUF first so the accumulator slot
                        # frees for the next head pair right away
                        nsb = rzpool.tile([D + 1, QC], F32, tag="nsb")
                        nc.vector.tensor_copy(nsb[:, :w], pattn[hp][:, :w])
                        rz = rzpool.tile([1, QC], F32, tag="rz")
                        nc.vector.reciprocal(rz[:, :w], nsb[D:D + 1, :w])
                        rzb = rzpool.tile([D, QC], F32, tag="rzb")
                        nc.gpsimd.partition_broadcast(rzb[:, :w], rz[0:1, :w])
                        nc.vector.tensor_mul(cT[hs, hc, :w],
                                             nsb[0:D, :w], rzb[:, :w])

            def emit_outproj_rs(ci, cT, q0, w):
                partial = dram.tile([w, E], F32, tag=f"partial{ci}")
                for t4 in range(w // P):
                    pso = [psA.tile([P, 512], F32, tag="psA",
                                    name=f"pso{eh}") for eh in range(2)]
                    for eh in range(2):
                        for fc in range(FH // P):
                            nc.tensor.matmul(
                                pso[eh][:],
                                lhsT=cT[:, fc, t4 * P:(t4 + 1) * P],
                                rhs=wo_sb[:, fc, eh * 512:(eh + 1) * 512],
                                start=(fc == 0), stop=(fc == FH // P - 1))
                    outsb = opool.tile([P, E], F32, tag="outsb")
                    for eh in range(2):
                        nc.vector.tensor_add(
                            outsb[:, eh * 512:(eh + 1) * 512],
                            pso[eh][:],
                            bo4b[:, eh * 512:(eh + 1) * 512])
                    nc.sync.dma_start(partial[t4 * P:(t4 + 1) * P, :],
                                      outsb[:])

                rsout = dram.tile([w // GROUPS, E], F32, tag=f"rsout{ci}")
                nc.gpsimd.collective_compute(
                    "ReduceScatter",
                    mybir.AluOpType.add,
                    replica_groups=[[0, 1, 2, 3], [4, 5, 6, 7]],
                    ins=[partial.opt()],
                    outs=[rsout.opt()],
                )
                nc.sync.dma_start(y.ap()[q0 // GROUPS:(q0 + w) // GROUPS, :],
                                  rsout[:])

            # ---- emission order: out-proj of chunk ci-1 is emitted after
            # attention of chunk ci, so its matmuls fill the PE-idle slots of
            # the ACT-bound attention inner loop ----
            qT0 = emit_qT(0, 0, CHUNKS[0])
            emit_phase1(0)
            pending = None  # (ci, cT, q0, w) awaiting out-proj
            q0 = 0
            for ci, w in enumerate(CHUNKS):
                qT = qT0 if ci == 0 else emit_qT(ci, q0, w)
                cT = cpool.tile([P, FH // P, QC], BF16, tag="cT",
                                name=f"cT{ci}")
                emit_attention(qT, cT, w,
                               mid=(lambda: emit_phase1(1)) if ci == 0 else None)
                if pending is not None:
                    emit_outproj_rs(*pending)
                pending = (ci, cT, q0, w)
                q0 += w
            emit_outproj_rs(*pending)

    nc.compile()
    return nc


def _get_nc():
    global _cached_nc
    if _cached_nc is None:
        _cached_nc = _build()
    return _cached_nc


def _in_maps(inputs, Wq, bq, Wk, bk, Wv, bv, Wo, bo):
    x = np.asarray(inputs, dtype=np.float32)
    xTb = [np.ascontiguousarray(x[b].T).astype(ml_dtypes.bfloat16)
           for b in range(B)]
    maps = []
    for c in range(N_CORES):
        b, g = divmod(c, GROUPS)
        fs, fe = g * FH, (g + 1) * FH
        maps.append({
            "xT": xTb[b],
            "wqT": np.ascontiguousarray(np.asarray(Wq)[fs:fe, :].T).astype(ml_dtypes.bfloat16),
            "wkT": np.ascontiguousarray(np.asarray(Wk)[fs:fe, :].T).astype(ml_dtypes.bfloat16),
            "wvT": np.ascontiguousarray(np.asarray(Wv)[fs:fe, :].T).astype(ml_dtypes.bfloat16),
            "woT": np.ascontiguousarray(np.asarray(Wo)[:, fs:fe].T).astype(ml_dtypes.bfloat16),
            "bq": np.ascontiguousarray(np.asarray(bq)[fs:fe], dtype=np.float32),
            "bk": np.ascontiguousarray(np.asarray(bk)[fs:fe], dtype=np.float32),
            "bv": np.ascontiguousarray(np.asarray(bv)[fs:fe], dtype=np.float32),
            "bo4": (np.asarray(bo, dtype=np.float32) / GROUPS).copy(),
        })
    return maps


def _assemble(results):
    out = np.empty((B, S, E), dtype=np.float32)
    for c in range(N_CORES):
        b, p = divmod(c, GROUPS)
        yc = results[c]["y"]  # [S//GROUPS, E]
        q0 = 0
        for w in CHUNKS:
            rows = w // GROUPS
            src = (q0 // GROUPS)
            out[b, q0 + p * rows:q0 + (p + 1) * rows, :] = yc[src:src + rows]
            q0 += w
    return out


def kernel(inputs, Wq, bq, Wk, bk, Wv, bv, Wo, bo, _run_kwargs=None):
    nc = _get_nc()
    maps = _in_maps(inputs, Wq, bq, Wk, bk, Wv, bv, Wo, bo)
    res = run_bass_kernel_spmd(nc, maps, core_ids=list(range(N_CORES)),
                               **(_run_kwargs or {}))
    if _run_kwargs:
        kernel.last_results = res
    return _assemble(res.results)
